# revision 1
# baseline (speedup 1.0000x reference)
"""LocalExpansion (7x7 unfold) Trainium2 Bass kernel.

Full input x: [2, 8, 2304, 64] f32 (B=2, heads=8, N=48*48, D=64).
Full output:  [2, 8, 2304, 49, 64] f32 — out[b,h,y*W+x,i*7+j,:] =
x_img[b,h,y+i-3,x+j-3,:] with zero fill outside the 48x48 image.

Strategy (pure DMA, memory-regime):
- batch*heads = 16 images, 2 per core across 8 NeuronCores.
- Per core: zero-pad each 48x48x64 image into SBUF as 54 rows
  (one padded row per partition, 54*64 floats = 13824 B). Image 0 on
  partitions 0-53 (even-SDMA-engine half), image 1 on partitions
  64-117 (odd half) so concurrent DMAs load all 16 SDMA engines.
- For each filter row i (7 of them) one 3D DMA writes the whole
  [48 y, 48 x, 7*64 floats] slab: src is an overlapping sliding
  window (x stride 64 floats < 448-float element) read from SBUF,
  dst is strided DRAM with 1792 B contiguous chunks. Boundary zeros
  come for free from the padded SBUF image.
HBM traffic per core = 57.8 MB writes + 1.2 MB reads (~roofline).
"""

import numpy as np

KH, KW = 7, 7
H, W, D = 48, 48, 64
PH, PW = H + 6, W + 6          # 54x54 padded image
ROW = PW * D                   # floats per padded row (one SBUF partition)
N = H * W                      # 2304
K = KH * KW                    # 49
IMG_OUT = N * K * D            # floats per image output
IMGS_PER_CORE = 2
N_CORES = 8
BASES = (0, 64)                # SBUF base partitions per image

_CACHE = {}


def _build_nc():
    import concourse.bass as bass
    import concourse.mybir as mybir

    nc = bass.Bass(trn_type="TRN2")
    x = nc.dram_tensor("x", [IMGS_PER_CORE, N, D], mybir.dt.float32,
                       kind="ExternalInput")
    out = nc.dram_tensor("out", [IMGS_PER_CORE, N, K, D], mybir.dt.float32,
                         kind="ExternalOutput")

    with (
        nc.sbuf_tensor("pad", [128, ROW], mybir.dt.float32) as pad,
        nc.semaphore("ld") as ld,
        nc.semaphore("ms") as ms,
        nc.semaphore("st") as st,
    ):
        # Zero the whole padded buffer once (pad strips stay zero), then
        # load both images into the padded interiors.
        nc.vector.memset(
            bass.AP(pad, 0, [[ROW, 128], [1, ROW]]), 0.0
        ).then_inc(ms, 1)
        nc.sync.wait_ge(ms, 1)
        for im in range(IMGS_PER_CORE):
            bp = BASES[im]
            nc.sync.dma_start(
                out=bass.AP(pad, (bp + 3) * ROW + 3 * D, [[ROW, H], [1, W * D]]),
                in_=bass.AP(x, im * N * D, [[W * D, H], [1, W * D]]),
            ).then_inc(ld, 16)

        nc.sync.wait_ge(ld, IMGS_PER_CORE * 16)
        nc.scalar.wait_ge(ld, IMGS_PER_CORE * 16)
        nc.scalar.wait_ge(ms, 1)

        # 7 filter-row slabs per image; interleave images so both SDMA
        # engine halves (even: partitions 0-63, odd: 64-127) stay busy,
        # and alternate the two HWDGE rings (sync/scalar) per i.
        n_st = 0
        for i in range(KH):
            ring = nc.sync if i % 2 == 0 else nc.scalar
            for im in range(IMGS_PER_CORE):
                bp = BASES[im]
                ring.dma_start(
                    out=bass.AP(
                        out,
                        im * IMG_OUT + i * KW * D,
                        [[W * K * D, H], [K * D, W], [1, KW * D]],
                    ),
                    in_=bass.AP(
                        pad,
                        (bp + i) * ROW,
                        [[ROW, H], [D, W], [1, KW * D]],
                    ),
                ).then_inc(st, 16)
                n_st += 16
        nc.sync.wait_ge(st, n_st)
        nc.scalar.wait_ge(st, n_st)
    return nc


def kernel(x, height=48, width=48):
    from concourse.bass_utils import run_bass_kernel_spmd

    x = np.asarray(x)
    b, nh = x.shape[0], x.shape[1]
    xi = np.ascontiguousarray(x.reshape(b * nh, N, D))
    in_maps = [
        {"x": np.ascontiguousarray(xi[IMGS_PER_CORE * c: IMGS_PER_CORE * (c + 1)])}
        for c in range(N_CORES)
    ]
    if "nc" not in _CACHE:
        _CACHE["nc"] = _build_nc()
    res = run_bass_kernel_spmd(_CACHE["nc"], in_maps, core_ids=list(range(N_CORES)))
    y = np.stack([res.results[c]["out"] for c in range(N_CORES)])
    return y.reshape(b, nh, N, K, D).astype(np.float32, copy=False)



# revision 3
# speedup vs baseline: 1.0827x; 1.0827x over previous
"""LocalExpansion (7x7 unfold) Trainium2 Bass kernel — DUP128 design.

Full input x: [2, 8, 2304, 64] f32 (B=2, heads=8, N=48*48, D=64).
Full output:  [2, 8, 2304, 49, 64] f32 — out[b,h,y*W+x,i*7+j,:] =
x_img[b,h,y+i-3,x+j-3,:] with zero fill outside the 48x48 image.

Strategy (memory-regime; descriptor-size + port-balance optimized):
- batch*heads = 16 images, 2 per core across 8 NeuronCores.
- Host preps per core a "7-row-window" staging of the input (im2col in
  y only, zero-padded): lane p in [0,96) = (im=p//48, y=p%48) holds
  rows y-3..y+3 restricted to window cols [0,42) (serves out cols
  0..35); lanes 96-127 are duplicate rows (rotating over 3 phases,
  window cols [36,54)) that serve out cols 36..47. All 128 SBUF
  partitions carry store traffic every tile -> all 16 SDMA ports
  balanced (96-lane layouts cap at 12/16 ports = ~240 GB/s).
- Per tile (G=4 columns, 9 tiles): DVE gathers the 49*64-float
  per-pixel blocks for 96 primary lanes, ACT for the 32 dup lanes,
  into a double-buffered exp tile [128, 4*3136]. One 3-4 DMA store
  writes 128 x 50176B contiguous descriptors (big descs ~ line rate;
  1792B sliding-window descs measured 2x slower).
- Dup rows 96-127 are reloaded between phases from precomputed DRAM
  (gpsimd/SWDGE ring, independent of the two HWDGE store rings).
HBM per core: 57.8 MB writes + 10.3 MB reads.
"""

import numpy as np

KH, KW = 7, 7
H, W, D = 48, 48, 64
N = H * W                       # 2304
K = KH * KW                     # 49
PXL = K * D                     # 3136 floats per output pixel
IMG_OUT = N * PXL               # floats per image output
IMGS_PER_CORE = 2
N_CORES = 8

G = 4                           # columns per tile
NT = 36 // G                    # 9 tiles (primary covers cols 0..35)
TPP = 12 // G                   # 3 tiles per phase (dup covers 12 cols)
PRIM_W = 42                     # primary window cols [0,42)
DUP_W = 18                      # dup window cols [36,54)
PSEG = PRIM_W * D               # 2688 floats per i-segment (primary)
DSEG = DUP_W * D                # 1152 floats per i-segment (dup)
PROW = KH * PSEG                # 18816 floats per primary lane
DROW = KH * DSEG                # 8064 floats per dup lane
EXPF = G * PXL                  # 12544 floats per lane per exp buffer

_CACHE = {}


def _build_nc():
    import concourse.bass as bass
    import concourse.mybir as mybir

    nc = bass.Bass(trn_type="TRN2")
    xp = nc.dram_tensor("xp", [96, PROW], mybir.dt.float32,
                        kind="ExternalInput")
    xd = nc.dram_tensor("xd", [3, 32, DROW], mybir.dt.float32,
                        kind="ExternalInput")
    out = nc.dram_tensor("out", [IMGS_PER_CORE, N, K, D], mybir.dt.float32,
                         kind="ExternalOutput")

    with (
        nc.sbuf_tensor("pad7", [128, PROW], mybir.dt.float32) as pad7,
        nc.sbuf_tensor("exp0", [128, EXPF], mybir.dt.float32) as exp0,
        nc.sbuf_tensor("exp1", [128, EXPF], mybir.dt.float32) as exp1,
        nc.semaphore("ld") as ld,
        nc.semaphore("rl") as rl,
        nc.semaphore("cp") as cp,
        nc.semaphore("st") as st,
    ):
        exps = (exp0, exp1)
        # Initial loads on the gpsimd (SWDGE) ring: primary windows to
        # parts 0-95, phase-0 dup windows to parts 96-127.
        nc.gpsimd.dma_start(
            out=bass.AP(pad7, 0, [[PROW, 96], [1, PROW]]),
            in_=bass.AP(xp, 0, [[PROW, 96], [1, PROW]]),
        ).then_inc(ld, 16)
        nc.gpsimd.dma_start(
            out=bass.AP(pad7, 96 * PROW, [[PROW, 32], [1, DROW]]),
            in_=bass.AP(xd, 0, [[DROW, 32], [1, DROW]]),
        ).then_inc(rl, 16)
        nc.vector.wait_ge(ld, 16)
        nc.scalar.wait_ge(rl, 16)

        # Phase reloads (queued up-front on gpsimd; each waits for the
        # last dup-copy of the previous phase to release parts 96-127).
        for ph in (1, 2):
            # dup copies done through tile (ph*TPP - 1): cp counts 2/tile
            nc.gpsimd.wait_ge(cp, 2 * ph * TPP)
            nc.gpsimd.dma_start(
                out=bass.AP(pad7, 96 * PROW, [[PROW, 32], [1, DROW]]),
                in_=bass.AP(xd, ph * 32 * DROW, [[DROW, 32], [1, DROW]]),
            ).then_inc(rl, 16)

        # Store DMA count per tile (for buffer-free waits).
        def tile_ndmas(t):
            ph = t // TPP
            return 2 + (1 if ph == 1 else 0)

        cum = [0]
        for t in range(NT):
            cum.append(cum[-1] + tile_ndmas(t))
        total_st = 16 * cum[-1]

        for t in range(NT):
            ph = t // TPP
            buf = exps[t % 2]
            ring = nc.sync if t % 2 == 0 else nc.scalar

            # Wait for the store that previously read this buffer.
            if t >= 2:
                nc.vector.wait_ge(st, 16 * cum[t - 1])
                nc.scalar.wait_ge(st, 16 * cum[t - 1])
            # ACT needs the dup content for this phase.
            if t % TPP == 0 and ph > 0:
                nc.scalar.wait_ge(rl, 16 * (ph + 1))

            # DVE: primary lanes 0-95, cols 4t..4t+3.
            nc.vector.tensor_copy(
                out=bass.AP(buf, 0,
                            [[EXPF, 96], [PXL, G], [KW * D, KH], [1, KW * D]]),
                in_=bass.AP(pad7, G * t * D,
                            [[PROW, 96], [D, G], [PSEG, KH], [1, KW * D]]),
            ).then_inc(cp, 1)
            # ACT: dup lanes 96-127, cols 36 + 4*(t%TPP) ...
            nc.scalar.copy(
                out=bass.AP(buf, 96 * EXPF,
                            [[EXPF, 32], [PXL, G], [KW * D, KH], [1, KW * D]]),
                in_=bass.AP(pad7, 96 * PROW + G * (t % TPP) * D,
                            [[PROW, 32], [D, G], [DSEG, KH], [1, KW * D]]),
            ).then_inc(cp, 1)

            ring.wait_ge(cp, 2 * (t + 1))
            # Primary store: 96 descs x 50176B.
            ring.dma_start(
                out=bass.AP(out, G * t * PXL,
                            [[IMG_OUT, 2], [W * PXL, H], [1, EXPF]]),
                in_=bass.AP(buf, 0, [[EXPF, 96], [1, EXPF]]),
            ).then_inc(st, 16)
            # Dup store(s): rows 32*ph..32*ph+31, cols 36+4*(t%TPP)..
            colbase = 36 + G * (t % TPP)
            if ph == 0:       # im0, y 0..31
                dsts = [(0, 0, 32)]
            elif ph == 1:     # im0 y32..47 + im1 y0..15
                dsts = [(0, 32, 16), (1, 0, 16)]
            else:             # im1, y 16..47
                dsts = [(1, 16, 32)]
            src_off = 96 * EXPF
            for im, y0, ny in dsts:
                ring.dma_start(
                    out=bass.AP(out, im * IMG_OUT + y0 * W * PXL
                                + colbase * PXL,
                                [[W * PXL, ny], [1, EXPF]]),
                    in_=bass.AP(buf, src_off, [[EXPF, ny], [1, EXPF]]),
                ).then_inc(st, 16)
                src_off += ny * EXPF

        nc.sync.wait_ge(st, total_st)
        nc.scalar.wait_ge(st, total_st)
        nc.gpsimd.wait_ge(st, total_st)
    return nc


def _in_maps_from_x(x):
    """Host prep: 7-row-window staging per core (see module docstring)."""
    x = np.asarray(x, dtype=np.float32)
    b, nh = x.shape[0], x.shape[1]
    img = np.ascontiguousarray(x.reshape(b * nh, H, W, D))
    in_maps = []
    for c in range(N_CORES):
        P = np.zeros((IMGS_PER_CORE, H + 6, W + 6, D), dtype=np.float32)
        P[:, 3:3 + H, 3:3 + W, :] = img[IMGS_PER_CORE * c:
                                        IMGS_PER_CORE * (c + 1)]
        # windows[im, y, i, xx, d] = P[im, y+i, xx, d]
        wins = np.stack([P[:, i:i + H] for i in range(KH)], axis=2)
        xp = np.ascontiguousarray(
            wins[:, :, :, 0:PRIM_W, :]).reshape(96, PROW)
        dup = np.ascontiguousarray(
            wins[:, :, :, 36:36 + DUP_W, :]).reshape(96, DROW)
        xd = dup.reshape(3, 32, DROW)
        in_maps.append({"xp": xp, "xd": xd})
    return in_maps


def kernel(x, height=48, width=48):
    from concourse.bass_utils import run_bass_kernel_spmd

    in_maps = _in_maps_from_x(x)
    if "nc" not in _CACHE:
        _CACHE["nc"] = _build_nc()
    res = run_bass_kernel_spmd(_CACHE["nc"], in_maps, core_ids=list(range(N_CORES)))
    y = np.stack([res.results[c]["out"] for c in range(N_CORES)])
    b, nh = np.asarray(x).shape[0], np.asarray(x).shape[1]
    return y.reshape(b, nh, N, K, D).astype(np.float32, copy=False)
